# revision 2
# baseline (speedup 1.0000x reference)
"""Trainium2 Bass kernel for a 3-layer GCN (nn_BalancedGCN).

Strategy (8 NeuronCores, graph/data parallel, dst-sharded):
  - Nodes partitioned into 8 blocks of 6250. Per layer, cores compute their
    slice of the projected+scaled feature table u = D^-1/2 (h W) (bf16 rows
    on a 256B stride), the host concatenates the slices between launches
    (device collectives deadlock against SWDGE custom DMA on this stack),
    and each core aggregates its 128-dst windows with dma_gather (the gpsimd
    MoE primitive, 128B payload elements) + a strided free-dim reduce on the
    Vector engine. The GCN norm dinv[src]*dinv[dst] factorizes through the
    gather: table rows are pre-scaled by dinv[src], window results
    post-scaled by dinv[dst] (fused into the ACT relu via its scale port).
  - dma_gather indices are signed int16, so the table is split at the
    core-aligned row 5*6272 into two gather structures per window. Within a
    core block, nodes are sorted by max(cnt0/b0, cnt1/b1) of their per-half
    in-degrees so both halves' per-window max degrees stay near-uniform
    (split padding 1.21x vs 1.70x for plain degree sort); per-edge work is
    dominated by Q7 descriptor generation (~6 ns/index), so padded index
    count is the primary cost.
  - Four launches total: P0 (x @ W1 projection), two mid layers (aggregate +
    relu + project), and the output layer (aggregate + bias).
"""
import os
import sys
import numpy as np

try:
    from concourse import bacc, bass, mybir, tile, bass_utils
except ImportError:  # pragma: no cover
    sys.path.insert(0, "/opt/trn_rl_repo")
    from concourse import bacc, bass, mybir, tile, bass_utils

from concourse import ap_utils
from concourse._compat import exact_div
from concourse.masks import make_identity

# Problem constants
N, E = 50000, 800000
F_IN, F_HID, F_OUT = 128, 64, 40
M = 8                      # cores
NL = N // M                # 6250 real nodes per core
P = 128
W = (NL + P - 1) // P      # 49 windows per core
NLP = W * P                # 6272 padded local slots
SPLIT = 5 * NLP            # table-half boundary (core-aligned, < 32768)
GMAX = 256                 # max gather-group columns (bf16: 32KB/partition)
TBL_W = 128                # physical table row width in bf16 (256B stride)

LAST_EXEC_NS = None
LAST_RESULTS = None


def dma_gather_raw(gp, out_ap, in_ap, idxs_ap, num_idxs, elem_size, elem_step,
                   queue_num=0):
    """nc.gpsimd.dma_gather without the client-side elem%256 assert.

    The Q7 ucode (dma_gather.cpp) only requires the row STRIDE to be a
    multiple of 256B (stride_bytes_256); the element byte length itself is a
    free descriptor length. Mirrors bass.BassGpSimd.dma_gather's lowering for
    the DRAM-source, non-transpose case.
    """
    assert idxs_ap.dtype == mybir.dt.int16
    assert in_ap.dtype == out_ap.dtype
    assert in_ap.space == bass.MemorySpace.DRAM
    assert idxs_ap.space == bass.MemorySpace.SBUF
    assert out_ap.space == bass.MemorySpace.SBUF
    assert ap_utils.ap_is_contiguous(out_ap.ap[1:])
    assert ap_utils.ap_is_contiguous(idxs_ap.ap[1:])
    assert in_ap.ap[-1][1] == out_ap.ap[-1][1] == elem_size
    assert out_ap.ap[0][1] * out_ap.ap[1][1] == ((num_idxs + 127) // 128) * 128
    assert in_ap.ap[0][0] == elem_step
    stride_bytes = elem_step * mybir.dt.size(in_ap.dtype)
    stride_bytes_256 = exact_div(stride_bytes, 256)
    assert 0 < stride_bytes_256 < 256
    _in_ap = gp.lower_ap_dma(in_ap, for_custom_bir_dma=True)
    inst = gp.add_instruction(
        mybir.InstDMAGatherAnt(
            name=gp.bass.get_next_instruction_name(),
            ins=[
                *_in_ap,
                gp.lower_ap(idxs_ap),
                gp.lower_val_access(gp.to_reg(num_idxs)),
            ],
            outs=[gp.lower_ap(out_ap)],
            transpose=False,
            num_idxs=num_idxs,
            elem_size=elem_size,
            stride_bytes_256=stride_bytes_256,
            gen_mode=0,
            single_packet=False,
            queue_num=queue_num,
            sbuf_tokens_per_rank=0,
            sbuf_free_dim_per_rank=0,
            sbuf_free_dim_pad_per_rank=0,
            sbuf_byte_offset=0,
        )
    )
    return inst


def _host_prep(edge_index):
    """Degree-sorted permutation + split padded column-major window indices."""
    src = np.asarray(edge_index[0], dtype=np.int64)
    dst = np.asarray(edge_index[1], dtype=np.int64)
    loops = np.arange(N, dtype=np.int64)
    src = np.concatenate([src, loops])
    dst = np.concatenate([dst, loops])
    deg = np.bincount(dst, minlength=N).astype(np.float64)
    dinv = np.where(deg > 0, 1.0 / np.sqrt(deg), 0.0).astype(np.float32)

    # sort nodes within each core block by the normalized max of their
    # per-table-half in-degrees: windows then have near-uniform D0 AND D1,
    # minimizing split padding (the SPLIT boundary is core-aligned, so
    # half membership is known before the within-core permutation).
    t0c = SPLIT // NLP
    beta0 = t0c / M
    t0 = (src // NL) < t0c
    cnt0g = np.bincount(dst[t0], minlength=N).astype(np.float64)
    cnt1g = deg - cnt0g
    key = np.maximum(cnt0g / beta0, cnt1g / (1.0 - beta0))

    perm_rows = np.empty(N, dtype=np.int64)
    orig_of = np.full((M, NLP), -1, dtype=np.int64)
    dinv_l = np.zeros((M, NLP), dtype=np.float32)
    for m in range(M):
        block = np.arange(m * NL, (m + 1) * NL)
        order = np.argsort(-key[block], kind="stable")
        nodes = block[order]
        perm_rows[nodes] = m * NLP + np.arange(NL)
        orig_of[m, :NL] = nodes
        dinv_l[m, :NL] = dinv[nodes]

    dst_row = perm_rows[dst]
    src_row = perm_rows[src]
    half = (src_row >= SPLIT).astype(np.int64)

    cnt = np.zeros((2, M, NLP), dtype=np.int64)
    np.add.at(cnt, (half, dst_row // NLP, dst_row % NLP), 1)
    D2 = cnt.reshape(2, M, W, P).max(axis=(1, 3)).astype(np.int64)  # [2, W]
    off0 = np.concatenate([[0], np.cumsum(D2[0])])
    off1 = np.concatenate([[0], np.cumsum(D2[1])])
    C0, C1 = int(off0[-1]), int(off1[-1])

    # zero pad rows inside each half (every core zeroes slots [NL, NLP))
    pad0 = NL
    k0 = (SPLIT + NLP - 1) // NLP
    pad1 = k0 * NLP + NL - SPLIT
    assert pad0 < SPLIT and 0 <= pad1 < M * NLP - SPLIT

    idx = [np.full((M, P, C0), pad0, dtype=np.int64),
           np.full((M, P, C1), pad1, dtype=np.int64)]
    # stable sort edges by (half, dst_row), then sequence within each group
    order = np.lexsort((np.arange(len(dst_row)), dst_row + half * 10 * M * NLP))
    hs = half[order]
    ds = dst_row[order]
    ss = src_row[order] - hs * SPLIT
    key = ds + hs * 10 * M * NLP
    uniq, first = np.unique(key, return_index=True)
    k = np.arange(len(ds)) - np.repeat(
        first, np.diff(np.concatenate([first, [len(ds)]]))
    )
    m_ = ds // NLP
    slot = ds % NLP
    offs = [off0, off1]
    for h in (0, 1):
        sel = hs == h
        idx[h][m_[sel], slot[sel] % P, offs[h][slot[sel] // P] + k[sel]] = ss[sel]
    return dict(
        dinv_l=dinv_l, idx0=idx[0], idx1=idx[1], D0=D2[0], D1=D2[1],
        off0=off0, off1=off1, C0=C0, C1=C1, orig_of=orig_of,
    )


def _wrap16(idx):
    """[P, C] logical indices -> [128, 8*C] int16 wrapped+replicated layout."""
    Pp, C = idx.shape
    assert Pp == 128
    w = idx.reshape(8, 16, C).transpose(1, 2, 0).reshape(16, 8 * C)
    return np.tile(w, (8, 1)).astype(np.int16)


def _pack_groups(D0, D1, off0, off1):
    """Greedy-pack windows into gather groups (<= GMAX total columns)."""
    groups = []
    lo = 0
    for w in range(W + 1):
        tot = (off0[w] - off0[lo]) + (off1[w] - off1[lo]) if w <= W else 0
        if w == W or (off0[w + 1] - off0[lo]) + (off1[w + 1] - off1[lo]) > GMAX:
            assert w > lo, "single window exceeds GMAX"
            groups.append((lo, w, int(off0[lo]), int(off0[w]),
                           int(off1[lo]), int(off1[w])))
            lo = w
    return groups


def _build_p0():
    """P0: u1 slice = dinv * (x @ W1), written to the 'u_out' table slice."""
    nc = bacc.Bacc("TRN2", target_bir_lowering=False, debug=False, num_devices=M)
    f32 = mybir.dt.float32
    bf16 = mybir.dt.bfloat16
    xT_ap = nc.dram_tensor("xT", [P, NLP], f32, kind="ExternalInput").ap()
    dinv_ap = nc.dram_tensor("dinv", [P, W], f32, kind="ExternalInput").ap()
    w1_ap = nc.dram_tensor("w1", [F_IN, F_HID], f32, kind="ExternalInput").ap()
    u_out = nc.dram_tensor("u_out", [NLP, TBL_W], bf16, kind="ExternalOutput").ap()
    with tile.TileContext(nc) as tc:
        with tc.tile_pool(name="cst", bufs=1) as cst, \
             tc.tile_pool(name="wrk", bufs=4) as wrk, \
             tc.tile_pool(name="psum", bufs=2, space="PSUM") as psum:
            dinv_t = cst.tile([P, W], f32)
            nc.sync.dma_start(out=dinv_t[:], in_=dinv_ap[:])
            w1_t = cst.tile([F_IN, F_HID], f32)
            nc.sync.dma_start(out=w1_t[:], in_=w1_ap[:])
            xT_t = cst.tile([P, NLP], f32)
            nc.sync.dma_start(out=xT_t[:], in_=xT_ap[:])
            for w in range(W):
                p_u1 = psum.tile([P, F_HID], f32, tag="p_u1")
                nc.tensor.matmul(
                    out=p_u1[:], lhsT=xT_t[:, w * P:(w + 1) * P],
                    rhs=w1_t[:], start=True, stop=True,
                )
                u1_s = wrk.tile([P, F_HID], bf16, tag="u1s")
                nc.vector.tensor_scalar_mul(u1_s[:], p_u1[:], dinv_t[:, w:w + 1])
                nc.sync.dma_start(out=u_out[w * P:(w + 1) * P, :F_HID], in_=u1_s[:])
    nc.compile()
    return nc


def _build_agg(prep, Fdim, mid):
    """Aggregation program: gather from replicated 'ufull', reduce windows.

    mid=True: t = relu(dinv*acc + b); u_out rows = ((dinv*t) @ Wn) (bf16).
    mid=False: out rows = dinv*acc + b (f32, F_OUT wide).
    """
    D0, D1, off0, off1, C0, C1 = (prep[k] for k in
                                  ["D0", "D1", "off0", "off1", "C0", "C1"])
    nc = bacc.Bacc("TRN2", target_bir_lowering=False, debug=False,
                   num_devices=M, num_swdge_queues=4)
    f32 = mybir.dt.float32
    bf16 = mybir.dt.bfloat16

    uf_ap = nc.dram_tensor("ufull", [M * NLP, TBL_W], bf16, kind="ExternalInput").ap()
    i0_ap = nc.dram_tensor("idx0", [P, 8 * C0], mybir.dt.int16, kind="ExternalInput").ap()
    i1_ap = nc.dram_tensor("idx1", [P, 8 * C1], mybir.dt.int16, kind="ExternalInput").ap()
    dinv_ap = nc.dram_tensor("dinv", [P, W], f32, kind="ExternalInput").ap()
    if mid:
        wn_ap = nc.dram_tensor("wn", [F_HID, F_HID], f32, kind="ExternalInput").ap()
        bb_ap = nc.dram_tensor("bb", [P, F_HID], f32, kind="ExternalInput").ap()
        out_ap = nc.dram_tensor("u_out", [NLP, TBL_W], bf16, kind="ExternalOutput").ap()
    else:
        bb_ap = nc.dram_tensor("bb", [P, F_OUT], f32, kind="ExternalInput").ap()
        out_ap = nc.dram_tensor("out", [NLP, F_OUT], f32, kind="ExternalOutput").ap()

    groups = _pack_groups(D0, D1, off0, off1)

    with tile.TileContext(nc) as tc:
        with tc.tile_pool(name="cst", bufs=1) as cst, \
             tc.tile_pool(name="gth", bufs=3) as gth, \
             tc.tile_pool(name="wrk", bufs=4) as wrk, \
             tc.tile_pool(name="psum", bufs=2, space="PSUM") as psum:

            i0_t = cst.tile([P, 8 * C0], mybir.dt.int16)
            nc.sync.dma_start(out=i0_t[:], in_=i0_ap[:])
            i1_t = cst.tile([P, 8 * C1], mybir.dt.int16)
            nc.sync.dma_start(out=i1_t[:], in_=i1_ap[:])
            dinv_t = cst.tile([P, W], f32)
            nc.sync.dma_start(out=dinv_t[:], in_=dinv_ap[:])
            if mid:
                wn_t = cst.tile([F_HID, F_HID], f32)
                nc.sync.dma_start(out=wn_t[:], in_=wn_ap[:])
                bb_t = cst.tile([P, F_HID], f32)
            else:
                bb_t = cst.tile([P, F_OUT], f32)
            nc.sync.dma_start(out=bb_t[:], in_=bb_ap[:])
            ident = cst.tile([P, P], f32)
            make_identity(nc, ident[:])

            def stage_mid(w, acc):
                # t = relu(dinv*acc + b); v = dinv*t  == relu(dinv*(...)) on ACT
                dinv_b = dinv_t[:, w:w + 1].to_broadcast([P, F_HID])
                t_t = wrk.tile([P, F_HID], f32, tag="t")
                nc.vector.tensor_tensor(out=t_t[:], in0=acc[:, :F_HID],
                                        in1=dinv_b, op=mybir.AluOpType.mult)
                nc.vector.tensor_tensor(out=t_t[:], in0=t_t[:], in1=bb_t[:],
                                        op=mybir.AluOpType.add)
                nc.scalar.activation(t_t[:], t_t[:],
                                     mybir.ActivationFunctionType.Relu,
                                     scale=dinv_t[:, w:w + 1])
                vT_p = psum.tile([F_HID, P], f32, tag="vT")
                nc.tensor.transpose(out=vT_p[:], in_=t_t[:], identity=ident[:])
                vT_s = wrk.tile([F_HID, P], f32, tag="vTs")
                nc.vector.tensor_copy(out=vT_s[:], in_=vT_p[:])
                u_p = psum.tile([P, F_HID], f32, tag="u_p")
                nc.tensor.matmul(out=u_p[:], lhsT=vT_s[:], rhs=wn_t[:],
                                 start=True, stop=True)
                u_s = wrk.tile([P, F_HID], bf16, tag="u_s")
                nc.vector.tensor_copy(out=u_s[:], in_=u_p[:])
                nc.sync.dma_start(out=out_ap[w * P:(w + 1) * P, :F_HID], in_=u_s[:])

            def stage_out(w, acc):
                o_t = wrk.tile([P, F_OUT], f32, tag="o")
                nc.vector.tensor_tensor(
                    out=o_t[:], in0=acc[:, :F_OUT],
                    in1=dinv_t[:, w:w + 1].to_broadcast([P, F_OUT]),
                    op=mybir.AluOpType.mult)
                nc.vector.tensor_tensor(out=o_t[:], in0=o_t[:], in1=bb_t[:],
                                        op=mybir.AluOpType.add)
                nc.sync.dma_start(out=out_ap[w * P:(w + 1) * P, :], in_=o_t[:])

            finish = stage_mid if mid else stage_out
            Fdim = F_HID if mid else F_OUT
            qctr = [0]
            for gi, (wlo, whi, c0lo, c0hi, c1lo, c1hi) in enumerate(groups):
                n0, n1 = c0hi - c0lo, c1hi - c1lo
                g_t = gth.tile([P, GMAX, Fdim], bf16, tag="g")
                if os.environ.get("BASS_GCN_NOGATHER", "") == "1":
                    nc.vector.memset(g_t[:], 0)
                    n0 = n1 = 0
                # cap each dma_gather at 64 columns (8192 idxs, 512+1 descs
                # per SDMA engine) to stay inside the SWDGE ring carveout.
                # queue 0 instructions hold the Pool engine for their full
                # desc-gen time (cpu pair 0-1 acks late), serializing dispatch
                # of everything behind them — use queues 1-3 only.
                CHUNK = 48
                for s in range(0, n0, CHUNK):
                    e = min(n0, s + CHUNK)
                    dma_gather_raw(
                        nc.gpsimd, g_t[:, s:e, :],
                        uf_ap[:SPLIT, :Fdim],
                        i0_t[:, 8 * (c0lo + s):8 * (c0lo + e)],
                        (e - s) * P, Fdim, TBL_W,
                        queue_num=1 + qctr[0] % 3,
                    )
                    qctr[0] += 1
                for s in range(0, n1, CHUNK):
                    e = min(n1, s + CHUNK)
                    dma_gather_raw(
                        nc.gpsimd, g_t[:, n0 + s:n0 + e, :],
                        uf_ap[SPLIT:, :Fdim],
                        i1_t[:, 8 * (c1lo + s):8 * (c1lo + e)],
                        (e - s) * P, Fdim, TBL_W,
                        queue_num=1 + qctr[0] % 3,
                    )
                    qctr[0] += 1
                for w in range(wlo, whi):
                    a0, b0 = int(off0[w]) - c0lo, int(off0[w + 1]) - c0lo
                    a1 = n0 + int(off1[w]) - c1lo
                    b1_ = n0 + int(off1[w + 1]) - c1lo
                    acc = wrk.tile([P, F_HID], f32, tag="acc")
                    nc.vector.tensor_reduce(
                        out=acc[:, :Fdim],
                        in_=g_t[:, a0:b0, :].rearrange("p c f -> p f c"),
                        axis=mybir.AxisListType.X, op=mybir.AluOpType.add,
                    )
                    if b1_ > a1:
                        acc2 = wrk.tile([P, F_HID], f32, tag="acc2")
                        nc.vector.tensor_reduce(
                            out=acc2[:, :Fdim],
                            in_=g_t[:, a1:b1_, :].rearrange("p c f -> p f c"),
                            axis=mybir.AxisListType.X, op=mybir.AluOpType.add,
                        )
                        nc.vector.tensor_tensor(
                            out=acc[:, :Fdim], in0=acc[:, :Fdim],
                            in1=acc2[:, :Fdim], op=mybir.AluOpType.add)
                    finish(w, acc)

    nc.compile()
    return nc


def kernel(x, edge_index, W1, b1, W2, b2, W3, b3):
    global LAST_EXEC_NS, LAST_RESULTS
    x = np.asarray(x, dtype=np.float32)
    W1 = np.asarray(W1, dtype=np.float32)
    b1 = np.asarray(b1, dtype=np.float32)
    W2 = np.asarray(W2, dtype=np.float32)
    b2 = np.asarray(b2, dtype=np.float32)
    W3 = np.asarray(W3, dtype=np.float32)
    b3 = np.asarray(b3, dtype=np.float32)

    prep = _host_prep(edge_index)
    orig_of = prep["orig_of"]

    nc0 = _build_p0()
    nc_mid = _build_agg(prep, F_HID, mid=True)
    nc_out = _build_agg(prep, F_OUT, mid=False)

    b1b = np.broadcast_to(b1, (P, F_HID)).copy()
    b2b = np.broadcast_to(b2, (P, F_HID)).copy()
    b3b = np.broadcast_to(b3, (P, F_OUT)).copy()
    W3z = np.zeros((F_HID, F_HID), np.float32)
    W3z[:, :F_OUT] = W3

    trace = os.environ.get("BASS_GCN_TRACE", "") == "1"
    if trace:
        bass_utils.upload_artifacts = lambda d: d
    cores = list(range(M))
    exec_ns = []

    def run(nc, in_maps):
        res = bass_utils.run_bass_kernel_spmd(nc, in_maps, core_ids=cores,
                                              trace=trace)
        if res.exec_time_ns is not None:
            exec_ns.append(res.exec_time_ns)
        return res.results

    dinv_m = [np.ascontiguousarray(prep["dinv_l"][m].reshape(W, P).T)
              for m in range(M)]
    i0_m = [_wrap16(prep["idx0"][m]) for m in range(M)]
    i1_m = [_wrap16(prep["idx1"][m]) for m in range(M)]

    # launch 0: u1 slices
    p0_maps = []
    for m in range(M):
        x_l = np.zeros((NLP, F_IN), np.float32)
        real = orig_of[m] >= 0
        x_l[real] = x[orig_of[m, real]]
        p0_maps.append({"xT": np.ascontiguousarray(x_l.T),
                        "dinv": dinv_m[m], "w1": W1})
    r0 = run(nc0, p0_maps)
    u_full = np.concatenate([r0[m]["u_out"] for m in range(M)], axis=0)

    # launch 1: layer-1 aggregation -> u2 slices
    r1 = run(nc_mid, [{"ufull": u_full, "idx0": i0_m[m], "idx1": i1_m[m],
                       "dinv": dinv_m[m], "wn": W2, "bb": b1b}
                      for m in range(M)])
    u_full = np.concatenate([r1[m]["u_out"] for m in range(M)], axis=0)

    # launch 2: layer-2 aggregation -> u3 slices (W3 zero-padded to 64)
    r2 = run(nc_mid, [{"ufull": u_full, "idx0": i0_m[m], "idx1": i1_m[m],
                       "dinv": dinv_m[m], "wn": W3z, "bb": b2b}
                      for m in range(M)])
    u_full = np.concatenate([r2[m]["u_out"] for m in range(M)], axis=0)

    # launch 3: layer-3 aggregation -> output rows
    r3 = run(nc_out, [{"ufull": u_full, "idx0": i0_m[m], "idx1": i1_m[m],
                       "dinv": dinv_m[m], "bb": b3b}
                      for m in range(M)])

    LAST_EXEC_NS = sum(exec_ns) if exec_ns else None
    LAST_RESULTS = exec_ns

    out = np.zeros((N, F_OUT), np.float32)
    for m in range(M):
        real = orig_of[m] >= 0
        out[orig_of[m, real]] = r3[m]["out"][:NLP][real]
    return out



# revision 3
# speedup vs baseline: 1.1581x; 1.1581x over previous
"""Trainium2 Bass kernel for a 3-layer GCN (nn_BalancedGCN).

Strategy (8 NeuronCores, graph/data parallel, dst-sharded):
  - Nodes partitioned into 8 blocks of 6250. Per layer, cores compute their
    slice of the projected+scaled feature table u = D^-1/2 (h W) (bf16 rows
    on a 256B stride), the host concatenates the slices between launches
    (device collectives deadlock against SWDGE custom DMA on this stack),
    and each core aggregates its 128-dst windows with dma_gather (the gpsimd
    MoE primitive, 128B payload elements) + a strided free-dim reduce on the
    Vector engine. The GCN norm dinv[src]*dinv[dst] factorizes through the
    gather: table rows are pre-scaled by dinv[src], window results
    post-scaled by dinv[dst] (fused into the ACT relu via its scale port).
  - dma_gather indices are signed int16, so the table is split at the
    core-aligned row 5*6272 into two gather structures per window. Within a
    core block, nodes are sorted by max(cnt0/b0, cnt1/b1) of their per-half
    in-degrees so both halves' per-window max degrees stay near-uniform
    (split padding 1.21x vs 1.70x for plain degree sort); per-edge work is
    dominated by Q7 descriptor generation (~6 ns/index), so padded index
    count is the primary cost.
  - Four launches total: P0 (x @ W1 projection), two mid layers (aggregate +
    relu + project), and the output layer (aggregate + bias).
"""
import os
import sys
import numpy as np

try:
    from concourse import bacc, bass, mybir, tile, bass_utils
except ImportError:  # pragma: no cover
    sys.path.insert(0, "/opt/trn_rl_repo")
    from concourse import bacc, bass, mybir, tile, bass_utils

from concourse import ap_utils
from concourse._compat import exact_div
from concourse.masks import make_identity

# Problem constants
N, E = 50000, 800000
F_IN, F_HID, F_OUT = 128, 64, 40
M = 8                      # cores
NL = N // M                # 6250 real nodes per core
P = 128
W = (NL + P - 1) // P      # 49 windows per core
NLP = W * P                # 6272 padded local slots
SPLIT = 5 * NLP            # table-half boundary (core-aligned, < 32768)
GMAX = 256                 # max gather-group columns (bf16: 32KB/partition)
TBL_W = 128                # physical table row width in bf16 (256B stride)

LAST_EXEC_NS = None
LAST_RESULTS = None


def dma_gather_raw(gp, out_ap, in_ap, idxs_ap, num_idxs, elem_size, elem_step,
                   queue_num=0):
    """nc.gpsimd.dma_gather without the client-side elem%256 assert.

    The Q7 ucode (dma_gather.cpp) only requires the row STRIDE to be a
    multiple of 256B (stride_bytes_256); the element byte length itself is a
    free descriptor length. Mirrors bass.BassGpSimd.dma_gather's lowering for
    the DRAM-source, non-transpose case.
    """
    assert idxs_ap.dtype == mybir.dt.int16
    assert in_ap.dtype == out_ap.dtype
    assert in_ap.space == bass.MemorySpace.DRAM
    assert idxs_ap.space == bass.MemorySpace.SBUF
    assert out_ap.space == bass.MemorySpace.SBUF
    assert ap_utils.ap_is_contiguous(out_ap.ap[1:])
    assert ap_utils.ap_is_contiguous(idxs_ap.ap[1:])
    assert in_ap.ap[-1][1] == out_ap.ap[-1][1] == elem_size
    assert out_ap.ap[0][1] * out_ap.ap[1][1] == ((num_idxs + 127) // 128) * 128
    assert in_ap.ap[0][0] == elem_step
    stride_bytes = elem_step * mybir.dt.size(in_ap.dtype)
    stride_bytes_256 = exact_div(stride_bytes, 256)
    assert 0 < stride_bytes_256 < 256
    _in_ap = gp.lower_ap_dma(in_ap, for_custom_bir_dma=True)
    inst = gp.add_instruction(
        mybir.InstDMAGatherAnt(
            name=gp.bass.get_next_instruction_name(),
            ins=[
                *_in_ap,
                gp.lower_ap(idxs_ap),
                gp.lower_val_access(gp.to_reg(num_idxs)),
            ],
            outs=[gp.lower_ap(out_ap)],
            transpose=False,
            num_idxs=num_idxs,
            elem_size=elem_size,
            stride_bytes_256=stride_bytes_256,
            gen_mode=0,
            single_packet=False,
            queue_num=queue_num,
            sbuf_tokens_per_rank=0,
            sbuf_free_dim_per_rank=0,
            sbuf_free_dim_pad_per_rank=0,
            sbuf_byte_offset=0,
        )
    )
    return inst


def _host_prep(edge_index):
    """Degree-sorted permutation + split padded column-major window indices."""
    src = np.asarray(edge_index[0], dtype=np.int64)
    dst = np.asarray(edge_index[1], dtype=np.int64)
    loops = np.arange(N, dtype=np.int64)
    src = np.concatenate([src, loops])
    dst = np.concatenate([dst, loops])
    deg = np.bincount(dst, minlength=N).astype(np.float64)
    dinv = np.where(deg > 0, 1.0 / np.sqrt(deg), 0.0).astype(np.float32)

    # sort nodes within each core block by the normalized max of their
    # per-table-half in-degrees: windows then have near-uniform D0 AND D1,
    # minimizing split padding (the SPLIT boundary is core-aligned, so
    # half membership is known before the within-core permutation).
    t0c = SPLIT // NLP
    beta0 = t0c / M
    t0 = (src // NL) < t0c
    cnt0g = np.bincount(dst[t0], minlength=N).astype(np.float64)
    cnt1g = deg - cnt0g
    key = np.maximum(cnt0g / beta0, cnt1g / (1.0 - beta0))

    perm_rows = np.empty(N, dtype=np.int64)
    orig_of = np.full((M, NLP), -1, dtype=np.int64)
    dinv_l = np.zeros((M, NLP), dtype=np.float32)
    for m in range(M):
        block = np.arange(m * NL, (m + 1) * NL)
        order = np.argsort(-key[block], kind="stable")
        nodes = block[order]
        perm_rows[nodes] = m * NLP + np.arange(NL)
        orig_of[m, :NL] = nodes
        dinv_l[m, :NL] = dinv[nodes]

    dst_row = perm_rows[dst]
    src_row = perm_rows[src]
    half = (src_row >= SPLIT).astype(np.int64)

    cnt = np.zeros((2, M, NLP), dtype=np.int64)
    np.add.at(cnt, (half, dst_row // NLP, dst_row % NLP), 1)
    D2 = cnt.reshape(2, M, W, P).max(axis=(1, 3)).astype(np.int64)  # [2, W]
    off0 = np.concatenate([[0], np.cumsum(D2[0])])
    off1 = np.concatenate([[0], np.cumsum(D2[1])])
    C0, C1 = int(off0[-1]), int(off1[-1])

    # zero pad rows inside each half (every core zeroes slots [NL, NLP))
    pad0 = NL
    k0 = (SPLIT + NLP - 1) // NLP
    pad1 = k0 * NLP + NL - SPLIT
    assert pad0 < SPLIT and 0 <= pad1 < M * NLP - SPLIT

    idx = [np.full((M, P, C0), pad0, dtype=np.int64),
           np.full((M, P, C1), pad1, dtype=np.int64)]
    # stable sort edges by (half, dst_row), then sequence within each group
    order = np.lexsort((np.arange(len(dst_row)), dst_row + half * 10 * M * NLP))
    hs = half[order]
    ds = dst_row[order]
    ss = src_row[order] - hs * SPLIT
    key = ds + hs * 10 * M * NLP
    uniq, first = np.unique(key, return_index=True)
    k = np.arange(len(ds)) - np.repeat(
        first, np.diff(np.concatenate([first, [len(ds)]]))
    )
    m_ = ds // NLP
    slot = ds % NLP
    offs = [off0, off1]
    for h in (0, 1):
        sel = hs == h
        idx[h][m_[sel], slot[sel] % P, offs[h][slot[sel] // P] + k[sel]] = ss[sel]
    return dict(
        dinv_l=dinv_l, idx0=idx[0], idx1=idx[1], D0=D2[0], D1=D2[1],
        off0=off0, off1=off1, C0=C0, C1=C1, orig_of=orig_of,
    )


def _wrap16(idx):
    """[P, C] logical indices -> [128, 8*C] int16 wrapped+replicated layout."""
    Pp, C = idx.shape
    assert Pp == 128
    w = idx.reshape(8, 16, C).transpose(1, 2, 0).reshape(16, 8 * C)
    return np.tile(w, (8, 1)).astype(np.int16)


def _pack_groups(D0, D1, off0, off1):
    """Greedy-pack windows into gather groups (<= GMAX total columns)."""
    groups = []
    lo = 0
    for w in range(W + 1):
        tot = (off0[w] - off0[lo]) + (off1[w] - off1[lo]) if w <= W else 0
        if w == W or (off0[w + 1] - off0[lo]) + (off1[w + 1] - off1[lo]) > GMAX:
            assert w > lo, "single window exceeds GMAX"
            groups.append((lo, w, int(off0[lo]), int(off0[w]),
                           int(off1[lo]), int(off1[w])))
            lo = w
    return groups


def _build_p0():
    """P0: u1 slice = dinv * (x @ W1), written to the 'u_out' table slice."""
    nc = bacc.Bacc("TRN2", target_bir_lowering=False, debug=False, num_devices=M)
    f32 = mybir.dt.float32
    bf16 = mybir.dt.bfloat16
    xT_ap = nc.dram_tensor("xT", [P, NLP], f32, kind="ExternalInput").ap()
    dinv_ap = nc.dram_tensor("dinv", [P, W], f32, kind="ExternalInput").ap()
    w1_ap = nc.dram_tensor("w1", [F_IN, F_HID], f32, kind="ExternalInput").ap()
    u_out = nc.dram_tensor("u_out", [NLP, TBL_W], bf16, kind="ExternalOutput").ap()
    with tile.TileContext(nc) as tc:
        with tc.tile_pool(name="cst", bufs=1) as cst, \
             tc.tile_pool(name="wrk", bufs=4) as wrk, \
             tc.tile_pool(name="psum", bufs=2, space="PSUM") as psum:
            dinv_t = cst.tile([P, W], f32)
            nc.sync.dma_start(out=dinv_t[:], in_=dinv_ap[:])
            w1_t = cst.tile([F_IN, F_HID], f32)
            nc.sync.dma_start(out=w1_t[:], in_=w1_ap[:])
            xT_t = cst.tile([P, NLP], f32)
            nc.sync.dma_start(out=xT_t[:], in_=xT_ap[:])
            for w in range(W):
                p_u1 = psum.tile([P, F_HID], f32, tag="p_u1")
                nc.tensor.matmul(
                    out=p_u1[:], lhsT=xT_t[:, w * P:(w + 1) * P],
                    rhs=w1_t[:], start=True, stop=True,
                )
                u1_s = wrk.tile([P, F_HID], bf16, tag="u1s")
                nc.vector.tensor_scalar_mul(u1_s[:], p_u1[:], dinv_t[:, w:w + 1])
                nc.sync.dma_start(out=u_out[w * P:(w + 1) * P, :F_HID], in_=u1_s[:])
    nc.compile()
    return nc


def _build_agg(prep, Fdim, mid):
    """Aggregation program: gather from replicated 'ufull', reduce windows.

    mid=True: t = relu(dinv*acc + b); u_out rows = ((dinv*t) @ Wn) (bf16).
    mid=False: out rows = dinv*acc + b (f32, F_OUT wide).
    """
    D0, D1, off0, off1, C0, C1 = (prep[k] for k in
                                  ["D0", "D1", "off0", "off1", "C0", "C1"])
    nc = bacc.Bacc("TRN2", target_bir_lowering=False, debug=False,
                   num_devices=M, num_swdge_queues=4)
    f32 = mybir.dt.float32
    bf16 = mybir.dt.bfloat16

    uf_ap = nc.dram_tensor("ufull", [M * NLP, TBL_W], bf16, kind="ExternalInput").ap()
    i0_ap = nc.dram_tensor("idx0", [P, 8 * C0], mybir.dt.int16, kind="ExternalInput").ap()
    i1_ap = nc.dram_tensor("idx1", [P, 8 * C1], mybir.dt.int16, kind="ExternalInput").ap()
    dinv_ap = nc.dram_tensor("dinv", [P, W], f32, kind="ExternalInput").ap()
    if mid:
        wn_ap = nc.dram_tensor("wn", [F_HID, F_HID], f32, kind="ExternalInput").ap()
        bb_ap = nc.dram_tensor("bb", [P, F_HID], f32, kind="ExternalInput").ap()
        out_ap = nc.dram_tensor("u_out", [NLP, TBL_W], bf16, kind="ExternalOutput").ap()
    else:
        bb_ap = nc.dram_tensor("bb", [P, F_OUT], f32, kind="ExternalInput").ap()
        out_ap = nc.dram_tensor("out", [NLP, F_OUT], f32, kind="ExternalOutput").ap()

    groups = _pack_groups(D0, D1, off0, off1)

    with tile.TileContext(nc) as tc:
        with tc.tile_pool(name="cst", bufs=1) as cst, \
             tc.tile_pool(name="gth", bufs=3) as gth, \
             tc.tile_pool(name="wrk", bufs=4) as wrk, \
             tc.tile_pool(name="psum", bufs=2, space="PSUM") as psum:

            i0_t = cst.tile([P, 8 * C0], mybir.dt.int16)
            nc.sync.dma_start(out=i0_t[:], in_=i0_ap[:])
            i1_t = cst.tile([P, 8 * C1], mybir.dt.int16)
            nc.sync.dma_start(out=i1_t[:], in_=i1_ap[:])
            dinv_t = cst.tile([P, W], f32)
            nc.sync.dma_start(out=dinv_t[:], in_=dinv_ap[:])
            if mid:
                wn_t = cst.tile([F_HID, F_HID], f32)
                nc.sync.dma_start(out=wn_t[:], in_=wn_ap[:])
                bb_t = cst.tile([P, F_HID], f32)
            else:
                bb_t = cst.tile([P, F_OUT], f32)
            nc.sync.dma_start(out=bb_t[:], in_=bb_ap[:])
            ident = cst.tile([P, P], f32)
            make_identity(nc, ident[:])

            def stage_mid(w, acc):
                # t = relu(dinv*acc + b); v = dinv*t  == relu(dinv*(...)) on ACT
                dinv_b = dinv_t[:, w:w + 1].to_broadcast([P, F_HID])
                t_t = wrk.tile([P, F_HID], f32, tag="t")
                nc.vector.tensor_tensor(out=t_t[:], in0=acc[:, :F_HID],
                                        in1=dinv_b, op=mybir.AluOpType.mult)
                nc.vector.tensor_tensor(out=t_t[:], in0=t_t[:], in1=bb_t[:],
                                        op=mybir.AluOpType.add)
                nc.scalar.activation(t_t[:], t_t[:],
                                     mybir.ActivationFunctionType.Relu,
                                     scale=dinv_t[:, w:w + 1])
                vT_p = psum.tile([F_HID, P], f32, tag="vT")
                nc.tensor.transpose(out=vT_p[:], in_=t_t[:], identity=ident[:])
                vT_s = wrk.tile([F_HID, P], f32, tag="vTs")
                nc.vector.tensor_copy(out=vT_s[:], in_=vT_p[:])
                u_p = psum.tile([P, F_HID], f32, tag="u_p")
                nc.tensor.matmul(out=u_p[:], lhsT=vT_s[:], rhs=wn_t[:],
                                 start=True, stop=True)
                u_s = wrk.tile([P, F_HID], bf16, tag="u_s")
                nc.vector.tensor_copy(out=u_s[:], in_=u_p[:])
                nc.sync.dma_start(out=out_ap[w * P:(w + 1) * P, :F_HID], in_=u_s[:])

            def stage_out(w, acc):
                o_t = wrk.tile([P, F_OUT], f32, tag="o")
                nc.vector.tensor_tensor(
                    out=o_t[:], in0=acc[:, :F_OUT],
                    in1=dinv_t[:, w:w + 1].to_broadcast([P, F_OUT]),
                    op=mybir.AluOpType.mult)
                nc.vector.tensor_tensor(out=o_t[:], in0=o_t[:], in1=bb_t[:],
                                        op=mybir.AluOpType.add)
                nc.sync.dma_start(out=out_ap[w * P:(w + 1) * P, :], in_=o_t[:])

            finish = stage_mid if mid else stage_out
            Fdim = F_HID if mid else F_OUT
            qctr = [0]
            for gi, (wlo, whi, c0lo, c0hi, c1lo, c1hi) in enumerate(groups):
                n0, n1 = c0hi - c0lo, c1hi - c1lo
                g_t = gth.tile([P, GMAX, Fdim], bf16, tag="g")
                if os.environ.get("BASS_GCN_NOGATHER", "") == "1":
                    nc.vector.memset(g_t[:], 0)
                    n0 = n1 = 0
                # cap each dma_gather at 64 columns (8192 idxs, 512+1 descs
                # per SDMA engine) to stay inside the SWDGE ring carveout.
                # queue 0 instructions hold the Pool engine for their full
                # desc-gen time (cpu pair 0-1 acks late), serializing dispatch
                # of everything behind them — use queues 1-3 only.
                CHUNK = 48
                for s in range(0, n0, CHUNK):
                    e = min(n0, s + CHUNK)
                    dma_gather_raw(
                        nc.gpsimd, g_t[:, s:e, :],
                        uf_ap[:SPLIT, :Fdim],
                        i0_t[:, 8 * (c0lo + s):8 * (c0lo + e)],
                        (e - s) * P, Fdim, TBL_W,
                        queue_num=(1, 2, 3, 0)[qctr[0] % 4],
                    )
                    qctr[0] += 1
                for s in range(0, n1, CHUNK):
                    e = min(n1, s + CHUNK)
                    dma_gather_raw(
                        nc.gpsimd, g_t[:, n0 + s:n0 + e, :],
                        uf_ap[SPLIT:, :Fdim],
                        i1_t[:, 8 * (c1lo + s):8 * (c1lo + e)],
                        (e - s) * P, Fdim, TBL_W,
                        queue_num=(1, 2, 3, 0)[qctr[0] % 4],
                    )
                    qctr[0] += 1
                for w in range(wlo, whi):
                    a0, b0 = int(off0[w]) - c0lo, int(off0[w + 1]) - c0lo
                    a1 = n0 + int(off1[w]) - c1lo
                    b1_ = n0 + int(off1[w + 1]) - c1lo
                    acc = wrk.tile([P, F_HID], f32, tag="acc")
                    nc.vector.tensor_reduce(
                        out=acc[:, :Fdim],
                        in_=g_t[:, a0:b0, :].rearrange("p c f -> p f c"),
                        axis=mybir.AxisListType.X, op=mybir.AluOpType.add,
                    )
                    if b1_ > a1:
                        acc2 = wrk.tile([P, F_HID], f32, tag="acc2")
                        nc.vector.tensor_reduce(
                            out=acc2[:, :Fdim],
                            in_=g_t[:, a1:b1_, :].rearrange("p c f -> p f c"),
                            axis=mybir.AxisListType.X, op=mybir.AluOpType.add,
                        )
                        nc.vector.tensor_tensor(
                            out=acc[:, :Fdim], in0=acc[:, :Fdim],
                            in1=acc2[:, :Fdim], op=mybir.AluOpType.add)
                    finish(w, acc)

    nc.compile()
    return nc


def kernel(x, edge_index, W1, b1, W2, b2, W3, b3):
    global LAST_EXEC_NS, LAST_RESULTS
    x = np.asarray(x, dtype=np.float32)
    W1 = np.asarray(W1, dtype=np.float32)
    b1 = np.asarray(b1, dtype=np.float32)
    W2 = np.asarray(W2, dtype=np.float32)
    b2 = np.asarray(b2, dtype=np.float32)
    W3 = np.asarray(W3, dtype=np.float32)
    b3 = np.asarray(b3, dtype=np.float32)

    prep = _host_prep(edge_index)
    orig_of = prep["orig_of"]

    nc0 = _build_p0()
    nc_mid = _build_agg(prep, F_HID, mid=True)
    nc_out = _build_agg(prep, F_OUT, mid=False)

    b1b = np.broadcast_to(b1, (P, F_HID)).copy()
    b2b = np.broadcast_to(b2, (P, F_HID)).copy()
    b3b = np.broadcast_to(b3, (P, F_OUT)).copy()
    W3z = np.zeros((F_HID, F_HID), np.float32)
    W3z[:, :F_OUT] = W3

    trace = os.environ.get("BASS_GCN_TRACE", "") == "1"
    if trace:
        bass_utils.upload_artifacts = lambda d: d
    cores = list(range(M))
    exec_ns = []

    def run(nc, in_maps):
        res = bass_utils.run_bass_kernel_spmd(nc, in_maps, core_ids=cores,
                                              trace=trace)
        if res.exec_time_ns is not None:
            exec_ns.append(res.exec_time_ns)
        return res.results

    dinv_m = [np.ascontiguousarray(prep["dinv_l"][m].reshape(W, P).T)
              for m in range(M)]
    i0_m = [_wrap16(prep["idx0"][m]) for m in range(M)]
    i1_m = [_wrap16(prep["idx1"][m]) for m in range(M)]

    # launch 0: u1 slices
    p0_maps = []
    for m in range(M):
        x_l = np.zeros((NLP, F_IN), np.float32)
        real = orig_of[m] >= 0
        x_l[real] = x[orig_of[m, real]]
        p0_maps.append({"xT": np.ascontiguousarray(x_l.T),
                        "dinv": dinv_m[m], "w1": W1})
    r0 = run(nc0, p0_maps)
    u_full = np.concatenate([r0[m]["u_out"] for m in range(M)], axis=0)

    # launch 1: layer-1 aggregation -> u2 slices
    r1 = run(nc_mid, [{"ufull": u_full, "idx0": i0_m[m], "idx1": i1_m[m],
                       "dinv": dinv_m[m], "wn": W2, "bb": b1b}
                      for m in range(M)])
    u_full = np.concatenate([r1[m]["u_out"] for m in range(M)], axis=0)

    # launch 2: layer-2 aggregation -> u3 slices (W3 zero-padded to 64)
    r2 = run(nc_mid, [{"ufull": u_full, "idx0": i0_m[m], "idx1": i1_m[m],
                       "dinv": dinv_m[m], "wn": W3z, "bb": b2b}
                      for m in range(M)])
    u_full = np.concatenate([r2[m]["u_out"] for m in range(M)], axis=0)

    # launch 3: layer-3 aggregation -> output rows
    r3 = run(nc_out, [{"ufull": u_full, "idx0": i0_m[m], "idx1": i1_m[m],
                       "dinv": dinv_m[m], "bb": b3b}
                      for m in range(M)])

    LAST_EXEC_NS = sum(exec_ns) if exec_ns else None
    LAST_RESULTS = exec_ns

    out = np.zeros((N, F_OUT), np.float32)
    for m in range(M):
        real = orig_of[m] >= 0
        out[orig_of[m, real]] = r3[m]["out"][:NLP][real]
    return out



# revision 4
# speedup vs baseline: 1.2226x; 1.0557x over previous
"""Trainium2 Bass kernel for a 3-layer GCN (nn_BalancedGCN).

Strategy (8 NeuronCores, graph/data parallel, dst-sharded):
  - Nodes partitioned into 8 blocks of 6250. Per layer, cores compute their
    slice of the projected+scaled feature table u = D^-1/2 (h W) (bf16 rows
    on a 256B stride), the host concatenates the slices between launches
    (device collectives deadlock against SWDGE custom DMA on this stack),
    and each core aggregates its 128-dst windows with dma_gather (the gpsimd
    MoE primitive, 128B payload elements) + a strided free-dim reduce on the
    Vector engine. The GCN norm dinv[src]*dinv[dst] factorizes through the
    gather: table rows are pre-scaled by dinv[src], window results
    post-scaled by dinv[dst] (fused into the ACT relu via its scale port).
  - dma_gather indices are signed int16, so the table is split at the
    core-aligned row 5*6272 into two gather structures per window. Within a
    core block, nodes are sorted by max(cnt0/b0, cnt1/b1) of their per-half
    in-degrees so both halves' per-window max degrees stay near-uniform
    (split padding 1.21x vs 1.70x for plain degree sort); per-edge work is
    dominated by Q7 descriptor generation (~6 ns/index), so padded index
    count is the primary cost.
  - Four launches total: P0 (x @ W1 projection), two mid layers (aggregate +
    relu + project), and the output layer (aggregate + bias).
"""
import os
import sys
import numpy as np

try:
    from concourse import bacc, bass, mybir, tile, bass_utils
except ImportError:  # pragma: no cover
    sys.path.insert(0, "/opt/trn_rl_repo")
    from concourse import bacc, bass, mybir, tile, bass_utils

from concourse import ap_utils
from concourse._compat import exact_div
from concourse.masks import make_identity

# Problem constants
N, E = 50000, 800000
F_IN, F_HID, F_OUT = 128, 64, 40
M = 8                      # cores
NL = N // M                # 6250 real nodes per core
P = 128
W = (NL + P - 1) // P      # 49 windows per core
NLP = W * P                # 6272 padded local slots
SPLIT = 5 * NLP            # table-half boundary (core-aligned, < 32768)
GMAX = 256                 # max gather-group columns (bf16: 32KB/partition)
TBL_W = 128                # physical table row width in bf16 (256B stride)

LAST_EXEC_NS = None
LAST_RESULTS = None


def dma_gather_raw(gp, out_ap, in_ap, idxs_ap, num_idxs, elem_size, elem_step,
                   queue_num=0):
    """nc.gpsimd.dma_gather without the client-side elem%256 assert.

    The Q7 ucode (dma_gather.cpp) only requires the row STRIDE to be a
    multiple of 256B (stride_bytes_256); the element byte length itself is a
    free descriptor length. Mirrors bass.BassGpSimd.dma_gather's lowering for
    the DRAM-source, non-transpose case.
    """
    assert idxs_ap.dtype == mybir.dt.int16
    assert in_ap.dtype == out_ap.dtype
    assert in_ap.space == bass.MemorySpace.DRAM
    assert idxs_ap.space == bass.MemorySpace.SBUF
    assert out_ap.space == bass.MemorySpace.SBUF
    assert ap_utils.ap_is_contiguous(out_ap.ap[1:])
    assert ap_utils.ap_is_contiguous(idxs_ap.ap[1:])
    assert in_ap.ap[-1][1] == out_ap.ap[-1][1] == elem_size
    assert out_ap.ap[0][1] * out_ap.ap[1][1] == ((num_idxs + 127) // 128) * 128
    assert in_ap.ap[0][0] == elem_step
    stride_bytes = elem_step * mybir.dt.size(in_ap.dtype)
    stride_bytes_256 = exact_div(stride_bytes, 256)
    assert 0 < stride_bytes_256 < 256
    _in_ap = gp.lower_ap_dma(in_ap, for_custom_bir_dma=True)
    inst = gp.add_instruction(
        mybir.InstDMAGatherAnt(
            name=gp.bass.get_next_instruction_name(),
            ins=[
                *_in_ap,
                gp.lower_ap(idxs_ap),
                gp.lower_val_access(gp.to_reg(num_idxs)),
            ],
            outs=[gp.lower_ap(out_ap)],
            transpose=False,
            num_idxs=num_idxs,
            elem_size=elem_size,
            stride_bytes_256=stride_bytes_256,
            gen_mode=0,
            single_packet=False,
            queue_num=queue_num,
            sbuf_tokens_per_rank=0,
            sbuf_free_dim_per_rank=0,
            sbuf_free_dim_pad_per_rank=0,
            sbuf_byte_offset=0,
        )
    )
    return inst


def _host_prep(edge_index):
    """Degree-sorted permutation + split padded column-major window indices."""
    src = np.asarray(edge_index[0], dtype=np.int64)
    dst = np.asarray(edge_index[1], dtype=np.int64)
    loops = np.arange(N, dtype=np.int64)
    src = np.concatenate([src, loops])
    dst = np.concatenate([dst, loops])
    deg = np.bincount(dst, minlength=N).astype(np.float64)
    dinv = np.where(deg > 0, 1.0 / np.sqrt(deg), 0.0).astype(np.float32)

    # sort nodes within each core block by the normalized max of their
    # per-table-half in-degrees: windows then have near-uniform D0 AND D1,
    # minimizing split padding (the SPLIT boundary is core-aligned, so
    # half membership is known before the within-core permutation).
    t0c = SPLIT // NLP
    beta0 = t0c / M
    t0 = (src // NL) < t0c
    cnt0g = np.bincount(dst[t0], minlength=N).astype(np.float64)
    cnt1g = deg - cnt0g
    key = np.maximum(cnt0g / beta0, cnt1g / (1.0 - beta0))

    perm_rows = np.empty(N, dtype=np.int64)
    orig_of = np.full((M, NLP), -1, dtype=np.int64)
    dinv_l = np.zeros((M, NLP), dtype=np.float32)
    for m in range(M):
        block = np.arange(m * NL, (m + 1) * NL)
        order = np.argsort(-key[block], kind="stable")
        nodes = block[order]
        perm_rows[nodes] = m * NLP + np.arange(NL)
        orig_of[m, :NL] = nodes
        dinv_l[m, :NL] = dinv[nodes]

    dst_row = perm_rows[dst]
    src_row = perm_rows[src]
    half = (src_row >= SPLIT).astype(np.int64)

    cnt = np.zeros((2, M, NLP), dtype=np.int64)
    np.add.at(cnt, (half, dst_row // NLP, dst_row % NLP), 1)
    D2 = cnt.reshape(2, M, W, P).max(axis=(1, 3)).astype(np.int64)  # [2, W]
    off0 = np.concatenate([[0], np.cumsum(D2[0])])
    off1 = np.concatenate([[0], np.cumsum(D2[1])])
    C0, C1 = int(off0[-1]), int(off1[-1])

    # zero pad rows inside each half (every core zeroes slots [NL, NLP))
    pad0 = NL
    k0 = (SPLIT + NLP - 1) // NLP
    pad1 = k0 * NLP + NL - SPLIT
    assert pad0 < SPLIT and 0 <= pad1 < M * NLP - SPLIT

    idx = [np.full((M, P, C0), pad0, dtype=np.int64),
           np.full((M, P, C1), pad1, dtype=np.int64)]
    # stable sort edges by (half, dst_row), then sequence within each group
    order = np.lexsort((np.arange(len(dst_row)), dst_row + half * 10 * M * NLP))
    hs = half[order]
    ds = dst_row[order]
    ss = src_row[order] - hs * SPLIT
    key = ds + hs * 10 * M * NLP
    uniq, first = np.unique(key, return_index=True)
    k = np.arange(len(ds)) - np.repeat(
        first, np.diff(np.concatenate([first, [len(ds)]]))
    )
    m_ = ds // NLP
    slot = ds % NLP
    offs = [off0, off1]
    for h in (0, 1):
        sel = hs == h
        idx[h][m_[sel], slot[sel] % P, offs[h][slot[sel] // P] + k[sel]] = ss[sel]
    return dict(
        dinv_l=dinv_l, idx0=idx[0], idx1=idx[1], D0=D2[0], D1=D2[1],
        off0=off0, off1=off1, C0=C0, C1=C1, orig_of=orig_of,
    )


def _wrap16(idx):
    """[P, C] logical indices -> [128, 8*C] int16 wrapped+replicated layout."""
    Pp, C = idx.shape
    assert Pp == 128
    w = idx.reshape(8, 16, C).transpose(1, 2, 0).reshape(16, 8 * C)
    return np.tile(w, (8, 1)).astype(np.int16)


def _pack_groups(D0, D1, off0, off1):
    """Greedy-pack windows into gather groups (<= GMAX total columns)."""
    groups = []
    lo = 0
    for w in range(W + 1):
        tot = (off0[w] - off0[lo]) + (off1[w] - off1[lo]) if w <= W else 0
        if w == W or (off0[w + 1] - off0[lo]) + (off1[w + 1] - off1[lo]) > GMAX:
            assert w > lo, "single window exceeds GMAX"
            groups.append((lo, w, int(off0[lo]), int(off0[w]),
                           int(off1[lo]), int(off1[w])))
            lo = w
    return groups


def _build_p0():
    """P0: u1 slice = dinv * (x @ W1), written to the 'u_out' table slice."""
    nc = bacc.Bacc("TRN2", target_bir_lowering=False, debug=False, num_devices=M)
    f32 = mybir.dt.float32
    bf16 = mybir.dt.bfloat16
    xT_ap = nc.dram_tensor("xT", [P, NLP], f32, kind="ExternalInput").ap()
    dinv_ap = nc.dram_tensor("dinv", [P, W], f32, kind="ExternalInput").ap()
    w1_ap = nc.dram_tensor("w1", [F_IN, F_HID], f32, kind="ExternalInput").ap()
    u_out = nc.dram_tensor("u_out", [NLP, TBL_W], bf16, kind="ExternalOutput").ap()
    with tile.TileContext(nc) as tc:
        with tc.tile_pool(name="cst", bufs=1) as cst, \
             tc.tile_pool(name="wrk", bufs=4) as wrk, \
             tc.tile_pool(name="psum", bufs=2, space="PSUM") as psum:
            dinv_t = cst.tile([P, W], f32)
            nc.sync.dma_start(out=dinv_t[:], in_=dinv_ap[:])
            w1_t = cst.tile([F_IN, F_HID], f32)
            nc.sync.dma_start(out=w1_t[:], in_=w1_ap[:])
            xT_t = cst.tile([P, NLP], f32)
            nc.sync.dma_start(out=xT_t[:], in_=xT_ap[:])
            for w in range(W):
                p_u1 = psum.tile([P, F_HID], f32, tag="p_u1")
                nc.tensor.matmul(
                    out=p_u1[:], lhsT=xT_t[:, w * P:(w + 1) * P],
                    rhs=w1_t[:], start=True, stop=True,
                )
                u1_s = wrk.tile([P, F_HID], bf16, tag="u1s")
                nc.vector.tensor_scalar_mul(u1_s[:], p_u1[:], dinv_t[:, w:w + 1])
                nc.sync.dma_start(out=u_out[w * P:(w + 1) * P, :F_HID], in_=u1_s[:])
    nc.compile()
    return nc


def _build_agg(prep, Fdim, mid):
    """Aggregation program: gather from replicated 'ufull', reduce windows.

    mid=True: t = relu(dinv*acc + b); u_out rows = ((dinv*t) @ Wn) (bf16).
    mid=False: out rows = dinv*acc + b (f32, F_OUT wide).
    """
    D0, D1, off0, off1, C0, C1 = (prep[k] for k in
                                  ["D0", "D1", "off0", "off1", "C0", "C1"])
    nc = bacc.Bacc("TRN2", target_bir_lowering=False, debug=False,
                   num_devices=M, num_swdge_queues=4)
    f32 = mybir.dt.float32
    bf16 = mybir.dt.bfloat16

    uf_ap = nc.dram_tensor("ufull", [M * NLP, TBL_W], bf16, kind="ExternalInput").ap()
    i0_ap = nc.dram_tensor("idx0", [P, 8 * C0], mybir.dt.int16, kind="ExternalInput").ap()
    i1_ap = nc.dram_tensor("idx1", [P, 8 * C1], mybir.dt.int16, kind="ExternalInput").ap()
    dinv_ap = nc.dram_tensor("dinv", [P, W], f32, kind="ExternalInput").ap()
    if mid:
        wn_ap = nc.dram_tensor("wn", [F_HID, F_HID], f32, kind="ExternalInput").ap()
        bb_ap = nc.dram_tensor("bb", [P, F_HID], f32, kind="ExternalInput").ap()
        out_ap = nc.dram_tensor("u_out", [NLP, TBL_W], bf16, kind="ExternalOutput").ap()
    else:
        bb_ap = nc.dram_tensor("bb", [P, F_OUT], f32, kind="ExternalInput").ap()
        out_ap = nc.dram_tensor("out", [NLP, F_OUT], f32, kind="ExternalOutput").ap()

    groups = _pack_groups(D0, D1, off0, off1)

    with tile.TileContext(nc) as tc:
        with tc.tile_pool(name="cst", bufs=1) as cst, \
             tc.tile_pool(name="gth", bufs=3) as gth, \
             tc.tile_pool(name="wrk", bufs=4) as wrk, \
             tc.tile_pool(name="psum", bufs=2, space="PSUM") as psum:

            i0_t = cst.tile([P, 8 * C0], mybir.dt.int16)
            nc.sync.dma_start(out=i0_t[:], in_=i0_ap[:])
            i1_t = cst.tile([P, 8 * C1], mybir.dt.int16)
            nc.sync.dma_start(out=i1_t[:], in_=i1_ap[:])
            dinv_t = cst.tile([P, W], f32)
            nc.sync.dma_start(out=dinv_t[:], in_=dinv_ap[:])
            if mid:
                wn_t = cst.tile([F_HID, F_HID], f32)
                nc.sync.dma_start(out=wn_t[:], in_=wn_ap[:])
                bb_t = cst.tile([P, F_HID], f32)
            else:
                bb_t = cst.tile([P, F_OUT], f32)
            nc.sync.dma_start(out=bb_t[:], in_=bb_ap[:])
            ident = cst.tile([P, P], f32)
            make_identity(nc, ident[:])

            def stage_mid(w, acc):
                # t = relu(dinv*acc + b); v = dinv*t  == relu(dinv*(...)) on ACT
                dinv_b = dinv_t[:, w:w + 1].to_broadcast([P, F_HID])
                t_t = wrk.tile([P, F_HID], f32, tag="t")
                nc.vector.tensor_tensor(out=t_t[:], in0=acc[:, :F_HID],
                                        in1=dinv_b, op=mybir.AluOpType.mult)
                nc.vector.tensor_tensor(out=t_t[:], in0=t_t[:], in1=bb_t[:],
                                        op=mybir.AluOpType.add)
                nc.scalar.activation(t_t[:], t_t[:],
                                     mybir.ActivationFunctionType.Relu,
                                     scale=dinv_t[:, w:w + 1])
                vT_p = psum.tile([F_HID, P], f32, tag="vT")
                nc.tensor.transpose(out=vT_p[:], in_=t_t[:], identity=ident[:])
                vT_s = wrk.tile([F_HID, P], f32, tag="vTs")
                nc.vector.tensor_copy(out=vT_s[:], in_=vT_p[:])
                u_p = psum.tile([P, F_HID], f32, tag="u_p")
                nc.tensor.matmul(out=u_p[:], lhsT=vT_s[:], rhs=wn_t[:],
                                 start=True, stop=True)
                u_s = wrk.tile([P, F_HID], bf16, tag="u_s")
                nc.vector.tensor_copy(out=u_s[:], in_=u_p[:])
                nc.sync.dma_start(out=out_ap[w * P:(w + 1) * P, :F_HID], in_=u_s[:])

            def stage_out(w, acc):
                o_t = wrk.tile([P, F_OUT], f32, tag="o")
                nc.vector.tensor_tensor(
                    out=o_t[:], in0=acc[:, :F_OUT],
                    in1=dinv_t[:, w:w + 1].to_broadcast([P, F_OUT]),
                    op=mybir.AluOpType.mult)
                nc.vector.tensor_tensor(out=o_t[:], in0=o_t[:], in1=bb_t[:],
                                        op=mybir.AluOpType.add)
                nc.sync.dma_start(out=out_ap[w * P:(w + 1) * P, :], in_=o_t[:])

            finish = stage_mid if mid else stage_out
            Fdim = F_HID if mid else F_OUT
            qctr = [0]
            for gi, (wlo, whi, c0lo, c0hi, c1lo, c1hi) in enumerate(groups):
                n0, n1 = c0hi - c0lo, c1hi - c1lo
                g_t = gth.tile([P, GMAX, Fdim], bf16, tag="g")
                if os.environ.get("BASS_GCN_NOGATHER", "") == "1":
                    nc.vector.memset(g_t[:], 0)
                    n0 = n1 = 0
                # cap each dma_gather at 64 columns (8192 idxs, 512+1 descs
                # per SDMA engine) to stay inside the SWDGE ring carveout.
                # queue 0 instructions hold the Pool engine for their full
                # desc-gen time (cpu pair 0-1 acks late), serializing dispatch
                # of everything behind them — use queues 1-3 only.
                CHUNK = 16
                for s in range(0, n0, CHUNK):
                    e = min(n0, s + CHUNK)
                    dma_gather_raw(
                        nc.gpsimd, g_t[:, s:e, :],
                        uf_ap[:SPLIT, :Fdim],
                        i0_t[:, 8 * (c0lo + s):8 * (c0lo + e)],
                        (e - s) * P, Fdim, TBL_W,
                        queue_num=(1, 2, 3, 0)[qctr[0] % 4],
                    )
                    qctr[0] += 1
                for s in range(0, n1, CHUNK):
                    e = min(n1, s + CHUNK)
                    dma_gather_raw(
                        nc.gpsimd, g_t[:, n0 + s:n0 + e, :],
                        uf_ap[SPLIT:, :Fdim],
                        i1_t[:, 8 * (c1lo + s):8 * (c1lo + e)],
                        (e - s) * P, Fdim, TBL_W,
                        queue_num=(1, 2, 3, 0)[qctr[0] % 4],
                    )
                    qctr[0] += 1
                for w in range(wlo, whi):
                    a0, b0 = int(off0[w]) - c0lo, int(off0[w + 1]) - c0lo
                    a1 = n0 + int(off1[w]) - c1lo
                    b1_ = n0 + int(off1[w + 1]) - c1lo
                    acc = wrk.tile([P, F_HID], f32, tag="acc")
                    nc.vector.tensor_reduce(
                        out=acc[:, :Fdim],
                        in_=g_t[:, a0:b0, :].rearrange("p c f -> p f c"),
                        axis=mybir.AxisListType.X, op=mybir.AluOpType.add,
                    )
                    if b1_ > a1:
                        acc2 = wrk.tile([P, F_HID], f32, tag="acc2")
                        nc.vector.tensor_reduce(
                            out=acc2[:, :Fdim],
                            in_=g_t[:, a1:b1_, :].rearrange("p c f -> p f c"),
                            axis=mybir.AxisListType.X, op=mybir.AluOpType.add,
                        )
                        nc.vector.tensor_tensor(
                            out=acc[:, :Fdim], in0=acc[:, :Fdim],
                            in1=acc2[:, :Fdim], op=mybir.AluOpType.add)
                    finish(w, acc)

    nc.compile()
    return nc


def kernel(x, edge_index, W1, b1, W2, b2, W3, b3):
    global LAST_EXEC_NS, LAST_RESULTS
    x = np.asarray(x, dtype=np.float32)
    W1 = np.asarray(W1, dtype=np.float32)
    b1 = np.asarray(b1, dtype=np.float32)
    W2 = np.asarray(W2, dtype=np.float32)
    b2 = np.asarray(b2, dtype=np.float32)
    W3 = np.asarray(W3, dtype=np.float32)
    b3 = np.asarray(b3, dtype=np.float32)

    prep = _host_prep(edge_index)
    orig_of = prep["orig_of"]

    nc0 = _build_p0()
    nc_mid = _build_agg(prep, F_HID, mid=True)
    nc_out = _build_agg(prep, F_OUT, mid=False)

    b1b = np.broadcast_to(b1, (P, F_HID)).copy()
    b2b = np.broadcast_to(b2, (P, F_HID)).copy()
    b3b = np.broadcast_to(b3, (P, F_OUT)).copy()
    W3z = np.zeros((F_HID, F_HID), np.float32)
    W3z[:, :F_OUT] = W3

    trace = os.environ.get("BASS_GCN_TRACE", "") == "1"
    if trace:
        bass_utils.upload_artifacts = lambda d: d
    cores = list(range(M))
    exec_ns = []

    def run(nc, in_maps):
        res = bass_utils.run_bass_kernel_spmd(nc, in_maps, core_ids=cores,
                                              trace=trace)
        if res.exec_time_ns is not None:
            exec_ns.append(res.exec_time_ns)
        return res.results

    dinv_m = [np.ascontiguousarray(prep["dinv_l"][m].reshape(W, P).T)
              for m in range(M)]
    i0_m = [_wrap16(prep["idx0"][m]) for m in range(M)]
    i1_m = [_wrap16(prep["idx1"][m]) for m in range(M)]

    # launch 0: u1 slices
    p0_maps = []
    for m in range(M):
        x_l = np.zeros((NLP, F_IN), np.float32)
        real = orig_of[m] >= 0
        x_l[real] = x[orig_of[m, real]]
        p0_maps.append({"xT": np.ascontiguousarray(x_l.T),
                        "dinv": dinv_m[m], "w1": W1})
    r0 = run(nc0, p0_maps)
    u_full = np.concatenate([r0[m]["u_out"] for m in range(M)], axis=0)

    # launch 1: layer-1 aggregation -> u2 slices
    r1 = run(nc_mid, [{"ufull": u_full, "idx0": i0_m[m], "idx1": i1_m[m],
                       "dinv": dinv_m[m], "wn": W2, "bb": b1b}
                      for m in range(M)])
    u_full = np.concatenate([r1[m]["u_out"] for m in range(M)], axis=0)

    # launch 2: layer-2 aggregation -> u3 slices (W3 zero-padded to 64)
    r2 = run(nc_mid, [{"ufull": u_full, "idx0": i0_m[m], "idx1": i1_m[m],
                       "dinv": dinv_m[m], "wn": W3z, "bb": b2b}
                      for m in range(M)])
    u_full = np.concatenate([r2[m]["u_out"] for m in range(M)], axis=0)

    # launch 3: layer-3 aggregation -> output rows
    r3 = run(nc_out, [{"ufull": u_full, "idx0": i0_m[m], "idx1": i1_m[m],
                       "dinv": dinv_m[m], "bb": b3b}
                      for m in range(M)])

    LAST_EXEC_NS = sum(exec_ns) if exec_ns else None
    LAST_RESULTS = exec_ns

    out = np.zeros((N, F_OUT), np.float32)
    for m in range(M):
        real = orig_of[m] >= 0
        out[orig_of[m, real]] = r3[m]["out"][:NLP][real]
    return out



# revision 6
# speedup vs baseline: 1.3560x; 1.1092x over previous
"""Trainium2 Bass kernel for a 3-layer GCN (nn_BalancedGCN).

Strategy (8 NeuronCores, graph/data parallel, dst-sharded):
  - Nodes partitioned into 8 blocks of 6250. Per layer, cores compute their
    slice of the projected+scaled feature table u = D^-1/2 (h W) (bf16 rows
    on a 256B stride), the host concatenates the slices between launches
    (device collectives deadlock against SWDGE custom DMA on this stack),
    and each core aggregates its 128-dst windows with dma_gather (the gpsimd
    MoE primitive) + a strided free-dim reduce on the Vector engine. The GCN
    norm dinv[src]*dinv[dst] factorizes through the gather: table rows are
    pre-scaled by dinv[src], window results post-scaled by dinv[dst] (fused
    into the ACT relu via its scale port).
  - dma_gather indices are signed int16, so the table is split at the
    core-aligned row 5*6272 into two gather structures per window. Within a
    core block, nodes are sorted by max(cnt0/b0, cnt1/b1) of their per-half
    in-degrees so both halves' per-window max degrees stay near-uniform.
  - Self-loops are excluded from the gather: the core's own table row for
    each dst slot is appended as one extra regular-DMA column per window
    (sequential 128-row read), which the window reduce then sums for free.
  - Per-(window, half) gather granularity with contiguous output columns:
    one tensor_reduce per window, a deep window-buffer rotation, gathers
    capped at CAP columns and rotated over SWDGE queues (1,2,3,0) — queue-0
    instructions hold the Pool engine for their full desc-gen (~8 ns/idx per
    queue pair), so they are dispatched last in each wave.
  - Four launches total: P0 (x @ W1 projection), two mid layers (aggregate +
    relu + project), and the output layer (aggregate + bias).
"""
import os
import sys
import numpy as np

try:
    from concourse import bacc, bass, mybir, tile, bass_utils
except ImportError:  # pragma: no cover
    sys.path.insert(0, "/opt/trn_rl_repo")
    from concourse import bacc, bass, mybir, tile, bass_utils

from concourse import ap_utils
from concourse._compat import exact_div
from concourse.masks import make_identity

# Problem constants
N, E = 50000, 800000
F_IN, F_HID, F_OUT = 128, 64, 40
M = 8                      # cores
NL = N // M                # 6250 real nodes per core
P = 128
W = (NL + P - 1) // P      # 49 windows per core
NLP = W * P                # 6272 padded local slots
SPLIT = 5 * NLP            # table-half boundary (core-aligned, < 32768)
TBL_W = 128                # physical table row width in bf16 (256B stride)
CAP = 24                   # max columns per dma_gather (3072 idxs)
GBUFS = 12                 # window buffers in flight

LAST_EXEC_NS = None
LAST_RESULTS = None


def dma_gather_raw(gp, out_ap, in_ap, idxs_ap, num_idxs, elem_size, elem_step,
                   queue_num=0):
    """nc.gpsimd.dma_gather without the client-side elem%256 assert.

    The Q7 ucode (dma_gather.cpp) only requires the row STRIDE to be a
    multiple of 256B (stride_bytes_256); the element byte length itself is a
    free descriptor length. Mirrors bass.BassGpSimd.dma_gather's lowering for
    the DRAM-source, non-transpose case.
    """
    assert idxs_ap.dtype == mybir.dt.int16
    assert in_ap.dtype == out_ap.dtype
    assert in_ap.space == bass.MemorySpace.DRAM
    assert idxs_ap.space == bass.MemorySpace.SBUF
    assert out_ap.space == bass.MemorySpace.SBUF
    assert ap_utils.ap_is_contiguous(out_ap.ap[1:])
    assert ap_utils.ap_is_contiguous(idxs_ap.ap[1:])
    assert in_ap.ap[-1][1] == out_ap.ap[-1][1] == elem_size
    assert out_ap.ap[0][1] * out_ap.ap[1][1] == ((num_idxs + 127) // 128) * 128
    assert in_ap.ap[0][0] == elem_step
    stride_bytes = elem_step * mybir.dt.size(in_ap.dtype)
    stride_bytes_256 = exact_div(stride_bytes, 256)
    assert 0 < stride_bytes_256 < 256
    _in_ap = gp.lower_ap_dma(in_ap, for_custom_bir_dma=True)
    inst = gp.add_instruction(
        mybir.InstDMAGatherAnt(
            name=gp.bass.get_next_instruction_name(),
            ins=[
                *_in_ap,
                gp.lower_ap(idxs_ap),
                gp.lower_val_access(gp.to_reg(num_idxs)),
            ],
            outs=[gp.lower_ap(out_ap)],
            transpose=False,
            num_idxs=num_idxs,
            elem_size=elem_size,
            stride_bytes_256=stride_bytes_256,
            gen_mode=0,
            single_packet=False,
            queue_num=queue_num,
            sbuf_tokens_per_rank=0,
            sbuf_free_dim_per_rank=0,
            sbuf_free_dim_pad_per_rank=0,
            sbuf_byte_offset=0,
        )
    )
    return inst


def _host_prep(edge_index):
    """Degree-sorted permutation + split padded column-major window indices.

    Self-loops are NOT in the edge lists (handled densely on device via the
    per-window self column); they DO count toward the degree normalization.
    """
    src = np.asarray(edge_index[0], dtype=np.int64)
    dst = np.asarray(edge_index[1], dtype=np.int64)
    deg = np.bincount(dst, minlength=N).astype(np.float64) + 1.0  # + self-loop
    dinv = (1.0 / np.sqrt(deg)).astype(np.float32)

    # sort nodes within each core block by the normalized max of their
    # per-table-half in-degrees: windows then have near-uniform D0 AND D1,
    # minimizing split padding (the SPLIT boundary is core-aligned, so
    # half membership is known before the within-core permutation).
    t0c = SPLIT // NLP
    beta0 = t0c / M
    t0 = (src // NL) < t0c
    cnt0g = np.bincount(dst[t0], minlength=N).astype(np.float64)
    cnt1g = np.bincount(dst[~t0], minlength=N).astype(np.float64)
    key = np.maximum(cnt0g / beta0, cnt1g / (1.0 - beta0))

    perm_rows = np.empty(N, dtype=np.int64)
    orig_of = np.full((M, NLP), -1, dtype=np.int64)
    dinv_l = np.zeros((M, NLP), dtype=np.float32)
    for m in range(M):
        block = np.arange(m * NL, (m + 1) * NL)
        order = np.argsort(-key[block], kind="stable")
        nodes = block[order]
        perm_rows[nodes] = m * NLP + np.arange(NL)
        orig_of[m, :NL] = nodes
        dinv_l[m, :NL] = dinv[nodes]

    dst_row = perm_rows[dst]
    src_row = perm_rows[src]
    half = (src_row >= SPLIT).astype(np.int64)

    cnt = np.zeros((2, M, NLP), dtype=np.int64)
    np.add.at(cnt, (half, dst_row // NLP, dst_row % NLP), 1)
    D2 = cnt.reshape(2, M, W, P).max(axis=(1, 3)).astype(np.int64)  # [2, W]
    off0 = np.concatenate([[0], np.cumsum(D2[0])])
    off1 = np.concatenate([[0], np.cumsum(D2[1])])
    C0, C1 = int(off0[-1]), int(off1[-1])

    # zero pad rows inside each half (every core zeroes slots [NL, NLP))
    pad0 = NL
    k0 = (SPLIT + NLP - 1) // NLP
    pad1 = k0 * NLP + NL - SPLIT
    assert pad0 < SPLIT and 0 <= pad1 < M * NLP - SPLIT

    idx = [np.full((M, P, C0), pad0, dtype=np.int64),
           np.full((M, P, C1), pad1, dtype=np.int64)]
    # stable sort edges by (half, dst_row), then sequence within each group
    order = np.lexsort((np.arange(len(dst_row)), dst_row + half * 10 * M * NLP))
    hs = half[order]
    ds = dst_row[order]
    ss = src_row[order] - hs * SPLIT
    key = ds + hs * 10 * M * NLP
    uniq, first = np.unique(key, return_index=True)
    k = np.arange(len(ds)) - np.repeat(
        first, np.diff(np.concatenate([first, [len(ds)]]))
    )
    m_ = ds // NLP
    slot = ds % NLP
    offs = [off0, off1]
    for h in (0, 1):
        sel = hs == h
        idx[h][m_[sel], slot[sel] % P, offs[h][slot[sel] // P] + k[sel]] = ss[sel]
    return dict(
        dinv_l=dinv_l, idx0=idx[0], idx1=idx[1], D0=D2[0], D1=D2[1],
        off0=off0, off1=off1, C0=C0, C1=C1, orig_of=orig_of,
    )


def _wrap16(idx):
    """[P, C] logical indices -> [128, 8*C] int16 wrapped+replicated layout."""
    Pp, C = idx.shape
    assert Pp == 128
    w = idx.reshape(8, 16, C).transpose(1, 2, 0).reshape(16, 8 * C)
    return np.tile(w, (8, 1)).astype(np.int16)


def _build_p0():
    """P0: u1 slice = dinv * (x @ W1), written to the 'u_out' table slice."""
    nc = bacc.Bacc("TRN2", target_bir_lowering=False, debug=False, num_devices=M)
    f32 = mybir.dt.float32
    bf16 = mybir.dt.bfloat16
    xT_ap = nc.dram_tensor("xT", [P, NLP], f32, kind="ExternalInput").ap()
    dinv_ap = nc.dram_tensor("dinv", [P, W], f32, kind="ExternalInput").ap()
    w1_ap = nc.dram_tensor("w1", [F_IN, F_HID], f32, kind="ExternalInput").ap()
    u_out = nc.dram_tensor("u_out", [NLP, TBL_W], bf16, kind="ExternalOutput").ap()
    with tile.TileContext(nc) as tc:
        with tc.tile_pool(name="cst", bufs=1) as cst, \
             tc.tile_pool(name="wrk", bufs=4) as wrk, \
             tc.tile_pool(name="psum", bufs=2, space="PSUM") as psum:
            dinv_t = cst.tile([P, W], f32)
            nc.sync.dma_start(out=dinv_t[:], in_=dinv_ap[:])
            w1_t = cst.tile([F_IN, F_HID], f32)
            nc.sync.dma_start(out=w1_t[:], in_=w1_ap[:])
            xT_t = cst.tile([P, NLP], f32)
            nc.sync.dma_start(out=xT_t[:], in_=xT_ap[:])
            for w in range(W):
                p_u1 = psum.tile([P, F_HID], f32, tag="p_u1")
                nc.tensor.matmul(
                    out=p_u1[:], lhsT=xT_t[:, w * P:(w + 1) * P],
                    rhs=w1_t[:], start=True, stop=True,
                )
                u1_s = wrk.tile([P, F_HID], bf16, tag="u1s")
                nc.vector.tensor_scalar_mul(u1_s[:], p_u1[:], dinv_t[:, w:w + 1])
                nc.sync.dma_start(out=u_out[w * P:(w + 1) * P, :F_HID], in_=u1_s[:])
    nc.compile()
    return nc


def _build_agg(prep, mid):
    """Aggregation program: per-window gathers + one reduce per window.

    mid=True: t = relu(dinv*acc + b); u_out rows = ((dinv*t) @ Wn) (bf16).
    mid=False: out rows = dinv*acc + b (f32, F_OUT wide).
    """
    D0, D1, off0, off1, C0, C1 = (prep[k] for k in
                                  ["D0", "D1", "off0", "off1", "C0", "C1"])
    nc = bacc.Bacc("TRN2", target_bir_lowering=False, debug=False,
                   num_devices=M, num_swdge_queues=4)
    f32 = mybir.dt.float32
    bf16 = mybir.dt.bfloat16
    Fdim = F_HID if mid else F_OUT
    DMAX = int((D0 + D1).max()) + 1  # +1 self column

    uf_ap = nc.dram_tensor("ufull", [M * NLP, TBL_W], bf16, kind="ExternalInput").ap()
    us_ap = nc.dram_tensor("uself", [NLP, TBL_W], bf16, kind="ExternalInput").ap()
    i0_ap = nc.dram_tensor("idx0", [P, 8 * C0], mybir.dt.int16, kind="ExternalInput").ap()
    i1_ap = nc.dram_tensor("idx1", [P, 8 * C1], mybir.dt.int16, kind="ExternalInput").ap()
    dinv_ap = nc.dram_tensor("dinv", [P, W], f32, kind="ExternalInput").ap()
    if mid:
        wn_ap = nc.dram_tensor("wn", [F_HID, F_HID], f32, kind="ExternalInput").ap()
        bb_ap = nc.dram_tensor("bb", [P, F_HID], f32, kind="ExternalInput").ap()
        out_ap = nc.dram_tensor("u_out", [NLP, TBL_W], bf16, kind="ExternalOutput").ap()
    else:
        bb_ap = nc.dram_tensor("bb", [P, F_OUT], f32, kind="ExternalInput").ap()
        out_ap = nc.dram_tensor("out", [NLP, F_OUT], f32, kind="ExternalOutput").ap()

    with tile.TileContext(nc) as tc:
        with tc.tile_pool(name="cst", bufs=1) as cst, \
             tc.tile_pool(name="gwin", bufs=GBUFS) as gwin, \
             tc.tile_pool(name="wrk", bufs=4) as wrk, \
             tc.tile_pool(name="psum", bufs=2, space="PSUM") as psum:

            i0_t = cst.tile([P, 8 * C0], mybir.dt.int16)
            nc.sync.dma_start(out=i0_t[:], in_=i0_ap[:])
            i1_t = cst.tile([P, 8 * C1], mybir.dt.int16)
            nc.sync.dma_start(out=i1_t[:], in_=i1_ap[:])
            dinv_t = cst.tile([P, W], f32)
            nc.sync.dma_start(out=dinv_t[:], in_=dinv_ap[:])
            if mid:
                wn_t = cst.tile([F_HID, F_HID], f32)
                nc.sync.dma_start(out=wn_t[:], in_=wn_ap[:])
                bb_t = cst.tile([P, F_HID], f32)
            else:
                bb_t = cst.tile([P, F_OUT], f32)
            nc.sync.dma_start(out=bb_t[:], in_=bb_ap[:])
            ident = cst.tile([P, P], f32)
            make_identity(nc, ident[:])

            def stage_mid(w, acc):
                # t = relu(dinv*acc + b); v = dinv*t  == relu(dinv*(...)) on ACT
                dinv_b = dinv_t[:, w:w + 1].to_broadcast([P, F_HID])
                t_t = wrk.tile([P, F_HID], f32, tag="t")
                nc.vector.tensor_tensor(out=t_t[:], in0=acc[:], in1=dinv_b,
                                        op=mybir.AluOpType.mult)
                nc.vector.tensor_tensor(out=t_t[:], in0=t_t[:], in1=bb_t[:],
                                        op=mybir.AluOpType.add)
                nc.scalar.activation(t_t[:], t_t[:],
                                     mybir.ActivationFunctionType.Relu,
                                     scale=dinv_t[:, w:w + 1])
                vT_p = psum.tile([F_HID, P], f32, tag="vT")
                nc.tensor.transpose(out=vT_p[:], in_=t_t[:], identity=ident[:])
                vT_s = wrk.tile([F_HID, P], f32, tag="vTs")
                nc.vector.tensor_copy(out=vT_s[:], in_=vT_p[:])
                u_p = psum.tile([P, F_HID], f32, tag="u_p")
                nc.tensor.matmul(out=u_p[:], lhsT=vT_s[:], rhs=wn_t[:],
                                 start=True, stop=True)
                u_s = wrk.tile([P, F_HID], bf16, tag="u_s")
                nc.vector.tensor_copy(out=u_s[:], in_=u_p[:])
                nc.sync.dma_start(out=out_ap[w * P:(w + 1) * P, :F_HID], in_=u_s[:])

            def stage_out(w, acc):
                o_t = wrk.tile([P, F_OUT], f32, tag="o")
                nc.vector.tensor_tensor(
                    out=o_t[:], in0=acc[:],
                    in1=dinv_t[:, w:w + 1].to_broadcast([P, F_OUT]),
                    op=mybir.AluOpType.mult)
                nc.vector.tensor_tensor(out=o_t[:], in0=o_t[:], in1=bb_t[:],
                                        op=mybir.AluOpType.add)
                nc.sync.dma_start(out=out_ap[w * P:(w + 1) * P, :], in_=o_t[:])

            finish = stage_mid if mid else stage_out
            qctr = [0]

            def gather_cols(g_t, gcol, half, clo, n):
                """Gather n table-half columns into g_t[:, gcol:gcol+n, :]."""
                if n == 0:
                    return
                base, hi, it = ((0, SPLIT, i0_t) if half == 0
                                else (SPLIT, M * NLP, i1_t))
                npieces = (n + CAP - 1) // CAP
                for i in range(npieces):
                    s = i * n // npieces
                    e = (i + 1) * n // npieces
                    dma_gather_raw(
                        nc.gpsimd, g_t[:, gcol + s:gcol + e, :],
                        uf_ap[base:hi, :Fdim],
                        it[:, 8 * (clo + s):8 * (clo + e)],
                        (e - s) * P, Fdim, TBL_W,
                        queue_num=(1, 2, 3, 0)[qctr[0] % 4],
                    )
                    qctr[0] += 1

            for w in range(W):
                n0, n1 = int(D0[w]), int(D1[w])
                cols = n0 + n1 + 1
                g_t = gwin.tile([P, DMAX, Fdim], bf16, tag="g")
                if os.environ.get("BASS_GCN_NOGATHER", "") == "1":
                    nc.vector.memset(g_t[:], 0)
                else:
                    gather_cols(g_t, 0, 0, int(off0[w]), n0)
                    gather_cols(g_t, n0, 1, int(off1[w]), n1)
                # self column: the core's own table rows for this window
                nc.sync.dma_start(
                    out=g_t[:, n0 + n1:cols, :],
                    in_=us_ap[w * P:(w + 1) * P, :Fdim].rearrange(
                        "p (c f) -> p c f", c=1),
                )
                acc = wrk.tile([P, Fdim], f32, tag="acc")
                nc.vector.tensor_reduce(
                    out=acc[:],
                    in_=g_t[:, :cols, :].rearrange("p c f -> p f c"),
                    axis=mybir.AxisListType.X, op=mybir.AluOpType.add,
                )
                finish(w, acc)

    nc.compile()
    return nc


def kernel(x, edge_index, W1, b1, W2, b2, W3, b3):
    global LAST_EXEC_NS, LAST_RESULTS
    x = np.asarray(x, dtype=np.float32)
    W1 = np.asarray(W1, dtype=np.float32)
    b1 = np.asarray(b1, dtype=np.float32)
    W2 = np.asarray(W2, dtype=np.float32)
    b2 = np.asarray(b2, dtype=np.float32)
    W3 = np.asarray(W3, dtype=np.float32)
    b3 = np.asarray(b3, dtype=np.float32)

    prep = _host_prep(edge_index)
    orig_of = prep["orig_of"]

    nc0 = _build_p0()
    nc_mid = _build_agg(prep, mid=True)
    nc_out = _build_agg(prep, mid=False)

    b1b = np.broadcast_to(b1, (P, F_HID)).copy()
    b2b = np.broadcast_to(b2, (P, F_HID)).copy()
    b3b = np.broadcast_to(b3, (P, F_OUT)).copy()
    W3z = np.zeros((F_HID, F_HID), np.float32)
    W3z[:, :F_OUT] = W3

    trace = os.environ.get("BASS_GCN_TRACE", "") == "1"
    if trace:
        bass_utils.upload_artifacts = lambda d: d
    cores = list(range(M))
    exec_ns = []

    def run(nc, in_maps):
        res = bass_utils.run_bass_kernel_spmd(nc, in_maps, core_ids=cores,
                                              trace=trace)
        if res.exec_time_ns is not None:
            exec_ns.append(res.exec_time_ns)
        return res.results

    dinv_m = [np.ascontiguousarray(prep["dinv_l"][m].reshape(W, P).T)
              for m in range(M)]
    i0_m = [_wrap16(prep["idx0"][m]) for m in range(M)]
    i1_m = [_wrap16(prep["idx1"][m]) for m in range(M)]

    # launch 0: u1 slices
    p0_maps = []
    for m in range(M):
        x_l = np.zeros((NLP, F_IN), np.float32)
        real = orig_of[m] >= 0
        x_l[real] = x[orig_of[m, real]]
        p0_maps.append({"xT": np.ascontiguousarray(x_l.T),
                        "dinv": dinv_m[m], "w1": W1})
    r0 = run(nc0, p0_maps)
    u_full = np.concatenate([r0[m]["u_out"] for m in range(M)], axis=0)

    # launch 1: layer-1 aggregation -> u2 slices
    r1 = run(nc_mid, [{"ufull": u_full,
                       "uself": u_full[m * NLP:(m + 1) * NLP],
                       "idx0": i0_m[m], "idx1": i1_m[m],
                       "dinv": dinv_m[m], "wn": W2, "bb": b1b}
                      for m in range(M)])
    u_full = np.concatenate([r1[m]["u_out"] for m in range(M)], axis=0)

    # launch 2: layer-2 aggregation -> u3 slices (W3 zero-padded to 64)
    r2 = run(nc_mid, [{"ufull": u_full,
                       "uself": u_full[m * NLP:(m + 1) * NLP],
                       "idx0": i0_m[m], "idx1": i1_m[m],
                       "dinv": dinv_m[m], "wn": W3z, "bb": b2b}
                      for m in range(M)])
    u_full = np.concatenate([r2[m]["u_out"] for m in range(M)], axis=0)

    # launch 3: layer-3 aggregation -> output rows
    r3 = run(nc_out, [{"ufull": u_full,
                       "uself": u_full[m * NLP:(m + 1) * NLP],
                       "idx0": i0_m[m], "idx1": i1_m[m],
                       "dinv": dinv_m[m], "bb": b3b}
                      for m in range(M)])

    LAST_EXEC_NS = sum(exec_ns) if exec_ns else None
    LAST_RESULTS = exec_ns

    out = np.zeros((N, F_OUT), np.float32)
    for m in range(M):
        real = orig_of[m] >= 0
        out[orig_of[m, real]] = r3[m]["out"][:NLP][real]
    return out
